# revision 36
# baseline (speedup 1.0000x reference)
"""GRU layer kernel for Trainium2, data-parallel over 8 NeuronCores.

Strategy (feature-major / weight-stationary, bf16 matmul path):
  - Shard batch N=2048 -> 8 cores x NB=256.
  - On host: arrange inputs per core as [T, p(128), k(KT), b(NB)] so each
    step's x load is one fully-contiguous DMA, and pre-pack each weight
    matrix W[dout,din] into the PE lhsT tile layout
    (tile (k,m): lhsT[p, q] = W[m*128+q, k*128+p]).
  - bf16 operands: full-speed PE (1 col/cycle) AND fast weight load (FWL
    reads 2 bf16/cycle; fp32 LDWEIGHTS gated the LDW+MM pair at ~120ns vs
    the 106.7ns matmul stream at FD=256). PSUM accumulation stays fp32.
  - On chip per timestep:
      z_pre.T = Wz_x| x_t.T  +  Wz_h| h.T      (8 K-tiles into PSUM)
      r_pre.T = likewise
      g_pre.T = Wh_x| x_t.T  +  Wh_h| (r*h).T
      z,r = sigmoid(+bias) on ACT per m-tile; g = tanh(+bias).
      DVE works on PSUM-bank pairs [128, 2*NB] (halves DVE op count).
    x-projection matmuls of step t+1 fill the PE while ACT/DVE resolve
    the recurrence.
  - Startup: staged DMA cascade matched to first-use order
    (wzx+x0 -> whx -> wrx -> wzh -> wrh -> whh), with dummy-matmul PE
    pre-warm so HAM un-throttles (1.2 -> 2.4 GHz) before real work.
  - Output written feature-major [T, D, NB] bf16 per core; host upcasts
    and transposes back.
"""
import os
import numpy as np

N, D = 2048, 512
T = int(os.environ.get("GRU_T", "64"))
NC = 8
NB = N // NC          # 256 batch rows per core
KT = D // 128         # 4 k-tiles
MT = D // 128         # 4 m-tiles

MM_DT = os.environ.get("GRU_MM_DT", "bf16")    # bf16 | fp32r | fp32

_CACHE = {}
LAST_RESULT = None


def _build_nc():
    import concourse.bacc as bacc
    import concourse.mybir as mybir
    from concourse.tile import TileContext

    f32 = mybir.dt.float32
    if MM_DT == "bf16":
        mdt = mybir.dt.bfloat16
    elif MM_DT == "fp32r":
        mdt = mybir.dt.float32r
    else:
        mdt = f32
    odt = mybir.dt.bfloat16 if MM_DT == "bf16" else f32
    # elementwise dtype: bf16 doubles DVE throughput (2 els/cycle/lane
    # when every operand is 16-bit); measured rel err stays ~1.4e-2 vs
    # the 2e-2 gate. The DVE blend chain is the step-boundary critical
    # path, so this buys back the last per-step stall and tail time.
    edt = mybir.dt.bfloat16 if MM_DT == "bf16" else f32
    Sig = mybir.ActivationFunctionType.Sigmoid
    Tanh = mybir.ActivationFunctionType.Tanh

    nc = bacc.Bacc("TRN2", target_bir_lowering=False, debug=False, num_devices=NC)

    xt_d = nc.dram_tensor("xt", [T, 128, KT * NB], mdt, kind="ExternalInput")
    w_d = {}
    for wname in ("wzx", "wzh", "wrx", "wrh", "whx", "whh"):
        w_d[wname] = nc.dram_tensor(wname, [128, KT * MT * 128], mdt, kind="ExternalInput")
    b_d = {}
    for bname in ("bz", "br", "bh"):
        b_d[bname] = nc.dram_tensor(bname, [128, MT], f32, kind="ExternalInput")
    out_d = nc.dram_tensor("out", [T, D, NB], odt, kind="ExternalOutput")

    with TileContext(nc) as tc:
        with (
            tc.tile_pool(name="wsb", bufs=1) as wsb,
            tc.tile_pool(name="xsb", bufs=6) as xsb,
            tc.tile_pool(name="ssb", bufs=2) as ssb,
            tc.tile_pool(name="hsb", bufs=3) as hsb,
            tc.tile_pool(name="psum", bufs=1, space="PSUM") as psum,
        ):
            w_sb = {}
            for wname in w_d:
                w_sb[wname] = wsb.tile([128, KT * MT * 128], mdt, name=f"w_{wname}")
            b_sb = {}
            for bname in b_d:
                b_sb[bname] = wsb.tile([128, MT], f32, name=f"b_{bname}")

            from concourse.tile import add_dep_helper

            # Startup DMAs: one dma_start per tensor, issued in
            # consumption order (wzx+x0 for t=0's z, whx for t=0's g,
            # then t=1's wrx/wzh/wrh/whh). Each transfer stripes across
            # all 16 DMA engines, and each engine drains its per-transfer
            # stripes in issue order — so issue order IS completion order
            # at full aggregate bandwidth. Semaphore-gating stages (tried)
            # serializes the ~600ns Sync-engine issues behind DMA
            # completions and strings the startup out 2-3x longer.
            def wdma(wname):
                return nc.sync.dma_start(out=w_sb[wname][:], in_=w_d[wname][:])

            xt_tiles = {}
            xdma_h = {}

            def get_xt(t):
                if t not in xt_tiles:
                    x = xsb.tile([128, KT * NB], mdt, name=f"x{t}", tag="xt")
                    xdma_h[t] = nc.sync.dma_start(out=x[:], in_=xt_d[t])
                    xt_tiles[t] = x
                return xt_tiles[t]

            # Early x tiles are interleaved between the weight DMAs: the
            # first matmuls of step t need xt(t), and an xt issued after
            # all six weights arrives ~8us too late (measured t0->t1 PE
            # gap waiting on the xt1 DMA semaphore).
            d_wzx = wdma("wzx")
            xt0 = get_xt(0)
            d_xt0 = xdma_h[0]
            wdma("whx")
            get_xt(1)
            nc.sync.dma_start(out=b_sb["bz"][:], in_=b_d["bz"][:])
            nc.sync.dma_start(out=b_sb["bh"][:], in_=b_d["bh"][:])
            wdma("wrx")
            wdma("wzh")
            get_xt(2)
            wdma("wrh")
            wdma("whh")
            get_xt(3)
            nc.sync.dma_start(out=b_sb["br"][:], in_=b_d["br"][:])

            # PE pre-warm: HAM leaves the PE at 1.2 GHz until ~3.4us of
            # sustained activity, and re-throttles after ~3.4us idle.
            # Dummy matmuls bridge the startup DMA window.
            warm_w = wsb.tile([128, 128], mdt, name="warm_w")
            nc.vector.memset(warm_w[:], 0.0)
            warm_ps = psum.tile([128, 512], f32, name="warm_ps", tag="warm")
            for i in range(30):
                nc.tensor.matmul(warm_ps[:, :128], warm_w[:], warm_w[:],
                                 start=True, stop=True)
            for gate_on in (d_wzx, d_xt0):
                for i in range(6):
                    wm = nc.tensor.matmul(warm_ps[:, :128], warm_w[:], warm_w[:],
                                          start=True, stop=True)
                    add_dep_helper(wm.ins, gate_on.ins, sync=True,
                                   reason="paced PE warmup")

            def wtile(wname, k, mi):
                off = (k * MT + mi) * 128
                return w_sb[wname][:, off:off + 128]

            def hview(h_m):
                # DVE can read bf16 directly; float32r tiles need a f32
                # bitcast for DVE consumption.
                return h_m[:] if MM_DT != "fp32r" else h_m[:].bitcast(f32)

            h_prev = [None] * MT

            # PSUM bank-pair sets, recreated per step via tags (6 data
            # banks + warm = 7 of 8). Accumulation groups are tracked per
            # BANK (zero region): exactly one start=True (first MM into
            # the bank) and one stop=True (last MM into the bank) even
            # though the two m-halves are separate output regions.
            zb_t, rb_t, gb_t = {}, {}, {}

            def half(banks, mi):
                return banks[mi // 2][:, (mi % 2) * NB:(mi % 2 + 1) * NB]

            def emit_x(t, gate_name):
                # x-projection block for one gate of step t (no recurrence
                # dependency; gated only by the weight DMA, the x tile DMA
                # and PSUM bank reuse)
                xt_t = get_xt(t)
                wname = {"z": "wzx", "r": "wrx", "g": "whx"}[gate_name]
                banks_map = {"z": zb_t, "r": rb_t, "g": gb_t}[gate_name]
                tag = {"z": "zb", "r": "rb", "g": "gb"}[gate_name]
                banks = [psum.tile([128, 512], f32, name=f"{gate_name}{t}_{i}",
                                   tag=f"{tag}{i}") for i in range(2)]
                banks_map[t] = banks
                last = t == 0 and gate_name in ("z", "g")
                for mi in range(MT):
                    for k in range(KT):
                        nc.tensor.matmul(half(banks, mi), wtile(wname, k, mi),
                                         xt_t[:, k * NB:(k + 1) * NB],
                                         start=(mi % 2 == 0 and k == 0),
                                         stop=(last and mi % 2 == 1 and k == KT - 1))

            emit_x(0, "z")
            emit_x(0, "g")

            for t in range(T):
                # x-projections for z(t) were emitted during step t-1
                # (between the Wrh and Whh blocks — they cover the PE's
                # ~1.1us wait for the r-gate ACT + r*h product)
                if t > 0:
                    emit_x(t, "r")
                    emit_x(t, "g")
                zb = zb_t.pop(t)
                gb = gb_t.pop(t)
                rb = rb_t.pop(t, None)

                # --- recurrent parts. Elementwise ops stay per m-tile
                # [128, NB]: the step-boundary critical path
                # (g ACT -> blend -> h -> next step's zh/rh matmuls) needs
                # h m-tiles to complete one at a time; [128, 2NB] pair ops
                # delayed h by ~0.5us/step (measured).
                z_t, g_t, h_t = [], [], []

                def emit_z_acts():
                    for mi in range(MT):
                        z_m = ssb.tile([128, NB], edt, name=f"z{t}m{mi}", tag=f"z{mi}")
                        nc.scalar.activation(z_m[:], half(zb, mi), Sig,
                                             bias=b_sb["bz"][:, mi:mi + 1])
                        z_t.append(z_m)

                if t > 0:
                    # k-outer: the k=0,1 matmuls for all m-tiles run first,
                    # giving the DVE blend of h m2/m3 (previous step) ~1us
                    # more slack before the k=2,3 matmuls consume them.
                    for k in range(KT):
                        for mi in range(MT):
                            nc.tensor.matmul(half(zb, mi), wtile("wzh", k, mi),
                                             h_prev[k][:],
                                             start=False,
                                             stop=(k == KT - 1 and mi % 2 == 1))
                    # z ACTs right away: they run during the Wrh block and
                    # free the zb banks for the hoisted x-z of step t+1
                    emit_z_acts()
                    for k in range(KT):
                        for mi in range(MT):
                            nc.tensor.matmul(half(rb, mi), wtile("wrh", k, mi),
                                             h_prev[k][:],
                                             start=False,
                                             stop=(k == KT - 1 and mi % 2 == 1))

                    # r gate first (feeds r*h -> Whh matmuls)
                    r_t, rh_t = [], []
                    for mi in range(MT):
                        r_m = ssb.tile([128, NB], edt, name=f"r{t}m{mi}", tag=f"r{mi}")
                        nc.scalar.activation(r_m[:], half(rb, mi), Sig,
                                             bias=b_sb["br"][:, mi:mi + 1])
                        r_t.append(r_m)
                    # Hoisted x-z of step t+1: in PE issue order this block
                    # sits between Wrh and Whh, covering the PE's ~1.1us
                    # wait for r ACT + r*h (measured ~190ns/step stall
                    # otherwise; GPSIMD rh muls made it worse).
                    if t + 1 < T:
                        emit_x(t + 1, "z")
                    for mi in range(MT):
                        rh_m = ssb.tile([128, NB], mdt, name=f"rh{t}m{mi}", tag=f"rh{mi}")
                        nc.vector.tensor_mul(rh_m[:], r_t[mi][:], hview(h_prev[mi]))
                        rh_t.append(rh_m)

                    for k in range(KT):
                        for mi in range(MT):
                            nc.tensor.matmul(half(gb, mi), wtile("whh", k, mi),
                                             rh_t[k][:],
                                             start=False,
                                             stop=(k == KT - 1 and mi % 2 == 1))
                else:
                    emit_z_acts()
                    if T > 1:
                        emit_x(1, "z")

                for mi in range(MT):
                    g_m = ssb.tile([128, NB], edt, name=f"g{t}m{mi}", tag=f"g{mi}")
                    nc.scalar.activation(g_m[:], half(gb, mi), Tanh,
                                         bias=b_sb["bh"][:, mi:mi + 1])
                    g_t.append(g_m)

                for mi in range(MT):
                    h_m = hsb.tile([128, NB], mdt, name=f"h{t}m{mi}", tag=f"h{mi}")
                    tmp = ssb.tile([128, NB], edt, name=f"tmp{t}m{mi}", tag=f"tmp{mi}")
                    # All blend chains stay on DVE. Offloading any of them
                    # (or the rh muls) to GPSIMD measured WORSE: its
                    # ~670ns ops + cross-engine semaphore latency feed the
                    # recurrence directly and stall the next step's
                    # matmuls far more than the ~244ns they were hiding.
                    if t == 0:
                        # h = (1 - z) * g = g - z*g
                        nc.vector.tensor_mul(tmp[:], z_t[mi][:], g_t[mi][:])
                        nc.vector.tensor_sub(h_m[:], g_t[mi][:], tmp[:])
                    else:
                        # h = g + z*(h_prev - g)
                        nc.vector.tensor_sub(tmp[:], hview(h_prev[mi]), g_t[mi][:])
                        nc.vector.tensor_mul(tmp[:], tmp[:], z_t[mi][:])
                        nc.vector.tensor_add(h_m[:], g_t[mi][:], tmp[:])
                    h_t.append(h_m)
                    nc.sync.dma_start(
                        out=out_d[t, mi * 128:(mi + 1) * 128, :],
                        in_=h_m[:] if MM_DT == "bf16" else h_m[:].bitcast(f32),
                    )
                h_prev = h_t

    nc.compile()
    return nc


def _get_nc():
    key = MM_DT
    if key not in _CACHE:
        _CACHE[key] = _build_nc()
    return _CACHE[key]


def _mm_np_dtype():
    if MM_DT == "bf16":
        import ml_dtypes
        return np.dtype(ml_dtypes.bfloat16)
    return np.dtype(np.float32)


def _pack_w(W):
    # W [dout, din] -> lhsT tiles packed [128, KT*MT*128], tile (k,m) at
    # free offset (k*MT+m)*128: w[p, off+q] = W[m*128+q, k*128+p]
    Wt = np.asarray(W, np.float32).T.reshape(KT, 128, MT, 128)
    packed = np.ascontiguousarray(Wt.transpose(1, 0, 2, 3).reshape(128, KT * MT * 128))
    return packed.astype(_mm_np_dtype())


def kernel(inputss, Wzx, Wzh, Wrx, Wrh, Whx, Whh, bz, br, bh):
    global LAST_RESULT
    from concourse.bass_utils import run_bass_kernel_spmd

    inputss = np.asarray(inputss, np.float32)
    assert inputss.shape == (N, T, D), inputss.shape

    # host-side shard + layout prep: [NC, T, p(128), k(KT), b(NB)] so the
    # per-step device DMA is fully contiguous
    xs = (inputss.reshape(NC, NB, T, KT, 128)
          .transpose(0, 2, 4, 3, 1)
          .reshape(NC, T, 128, KT * NB))
    wp = {"wzx": _pack_w(Wzx), "wzh": _pack_w(Wzh),
          "wrx": _pack_w(Wrx), "wrh": _pack_w(Wrh),
          "whx": _pack_w(Whx), "whh": _pack_w(Whh)}
    bp = {"bz": np.ascontiguousarray(np.asarray(bz, np.float32).reshape(MT, 128).T),
          "br": np.ascontiguousarray(np.asarray(br, np.float32).reshape(MT, 128).T),
          "bh": np.ascontiguousarray(np.asarray(bh, np.float32).reshape(MT, 128).T)}

    mmdt = _mm_np_dtype()
    in_maps = []
    for c in range(NC):
        m = {"xt": np.ascontiguousarray(xs[c]).astype(mmdt)}
        m.update(wp)
        m.update(bp)
        in_maps.append(m)

    nc = _get_nc()
    trace = bool(int(os.environ.get("GRU_TRACE", "0")))
    res = run_bass_kernel_spmd(nc, in_maps, core_ids=list(range(NC)), trace=trace)
    LAST_RESULT = res

    outs = np.stack([np.asarray(res.results[c]["out"], np.float32)
                     for c in range(NC)])  # [NC, T, D, NB]
    return np.ascontiguousarray(outs.transpose(0, 3, 1, 2).reshape(N, T, D))


# revision 37
# speedup vs baseline: 1.1773x; 1.1773x over previous
"""GRU layer kernel for Trainium2, data-parallel over 8 NeuronCores.

Strategy (feature-major / weight-stationary, bf16 matmul path):
  - Shard batch N=2048 -> 8 cores x NB=256.
  - On host: arrange inputs per core as [T, p(128), k(KT), b(NB)] so each
    step's x load is one fully-contiguous DMA, and pre-pack each weight
    matrix W[dout,din] into the PE lhsT tile layout
    (tile (k,m): lhsT[p, q] = W[m*128+q, k*128+p]).
  - bf16 operands: full-speed PE (1 col/cycle) AND fast weight load (FWL
    reads 2 bf16/cycle; fp32 LDWEIGHTS gated the LDW+MM pair at ~120ns vs
    the 106.7ns matmul stream at FD=256). PSUM accumulation stays fp32.
  - On chip per timestep:
      z_pre.T = Wz_x| x_t.T  +  Wz_h| h.T      (8 K-tiles into PSUM)
      r_pre.T = likewise
      g_pre.T = Wh_x| x_t.T  +  Wh_h| (r*h).T
      z,r = sigmoid(+bias) on ACT per m-tile; g = tanh(+bias).
      DVE works on PSUM-bank pairs [128, 2*NB] (halves DVE op count).
    x-projection matmuls of step t+1 fill the PE while ACT/DVE resolve
    the recurrence.
  - Startup: staged DMA cascade matched to first-use order
    (wzx+x0 -> whx -> wrx -> wzh -> wrh -> whh), with dummy-matmul PE
    pre-warm so HAM un-throttles (1.2 -> 2.4 GHz) before real work.
  - Output written feature-major [T, D, NB] bf16 per core; host upcasts
    and transposes back.
"""
import os
import numpy as np

N, D = 2048, 512
T = int(os.environ.get("GRU_T", "64"))
NC = 8
NB = N // NC          # 256 batch rows per core
KT = D // 128         # 4 k-tiles
MT = D // 128         # 4 m-tiles

MM_DT = os.environ.get("GRU_MM_DT", "bf16")    # bf16 | fp32r | fp32

_CACHE = {}
LAST_RESULT = None


def _build_nc():
    import concourse.bacc as bacc
    import concourse.mybir as mybir
    from concourse.tile import TileContext

    f32 = mybir.dt.float32
    if MM_DT == "bf16":
        mdt = mybir.dt.bfloat16
    elif MM_DT == "fp32r":
        mdt = mybir.dt.float32r
    else:
        mdt = f32
    odt = mybir.dt.bfloat16 if MM_DT == "bf16" else f32
    # NOTE: bf16 elementwise tiles (z/r/g/tmp) measured 822us vs 697us —
    # 16-bit does NOT speed these DVE/ACT ops up in practice; keep f32.
    Sig = mybir.ActivationFunctionType.Sigmoid
    Tanh = mybir.ActivationFunctionType.Tanh

    nc = bacc.Bacc("TRN2", target_bir_lowering=False, debug=False, num_devices=NC)

    xt_d = nc.dram_tensor("xt", [T, 128, KT * NB], mdt, kind="ExternalInput")
    w_d = {}
    for wname in ("wzx", "wzh", "wrx", "wrh", "whx", "whh"):
        w_d[wname] = nc.dram_tensor(wname, [128, KT * MT * 128], mdt, kind="ExternalInput")
    b_d = {}
    for bname in ("bz", "br", "bh"):
        b_d[bname] = nc.dram_tensor(bname, [128, MT], f32, kind="ExternalInput")
    out_d = nc.dram_tensor("out", [T, D, NB], odt, kind="ExternalOutput")

    with TileContext(nc) as tc:
        with (
            tc.tile_pool(name="wsb", bufs=1) as wsb,
            tc.tile_pool(name="xsb", bufs=6) as xsb,
            tc.tile_pool(name="ssb", bufs=2) as ssb,
            tc.tile_pool(name="hsb", bufs=3) as hsb,
            tc.tile_pool(name="psum", bufs=1, space="PSUM") as psum,
        ):
            w_sb = {}
            for wname in w_d:
                w_sb[wname] = wsb.tile([128, KT * MT * 128], mdt, name=f"w_{wname}")
            b_sb = {}
            for bname in b_d:
                b_sb[bname] = wsb.tile([128, MT], f32, name=f"b_{bname}")

            from concourse.tile import add_dep_helper

            # Startup DMAs: one dma_start per tensor, issued in
            # consumption order (wzx+x0 for t=0's z, whx for t=0's g,
            # then t=1's wrx/wzh/wrh/whh). Each transfer stripes across
            # all 16 DMA engines, and each engine drains its per-transfer
            # stripes in issue order — so issue order IS completion order
            # at full aggregate bandwidth. Semaphore-gating stages (tried)
            # serializes the ~600ns Sync-engine issues behind DMA
            # completions and strings the startup out 2-3x longer.
            def wdma(wname):
                return nc.sync.dma_start(out=w_sb[wname][:], in_=w_d[wname][:])

            xt_tiles = {}
            xdma_h = {}

            def get_xt(t):
                if t not in xt_tiles:
                    x = xsb.tile([128, KT * NB], mdt, name=f"x{t}", tag="xt")
                    xdma_h[t] = nc.sync.dma_start(out=x[:], in_=xt_d[t])
                    xt_tiles[t] = x
                return xt_tiles[t]

            # Early x tiles are interleaved between the weight DMAs: the
            # first matmuls of step t need xt(t), and an xt issued after
            # all six weights arrives ~8us too late (measured t0->t1 PE
            # gap waiting on the xt1 DMA semaphore).
            d_wzx = wdma("wzx")
            xt0 = get_xt(0)
            d_xt0 = xdma_h[0]
            wdma("whx")
            get_xt(1)
            nc.sync.dma_start(out=b_sb["bz"][:], in_=b_d["bz"][:])
            nc.sync.dma_start(out=b_sb["bh"][:], in_=b_d["bh"][:])
            wdma("wrx")
            wdma("wzh")
            get_xt(2)
            wdma("wrh")
            wdma("whh")
            get_xt(3)
            nc.sync.dma_start(out=b_sb["br"][:], in_=b_d["br"][:])

            # PE pre-warm: HAM leaves the PE at 1.2 GHz until ~3.4us of
            # sustained activity, and re-throttles after ~3.4us idle.
            # Dummy matmuls bridge the startup DMA window.
            warm_w = wsb.tile([128, 128], mdt, name="warm_w")
            nc.vector.memset(warm_w[:], 0.0)
            warm_ps = psum.tile([128, 512], f32, name="warm_ps", tag="warm")
            for i in range(30):
                nc.tensor.matmul(warm_ps[:, :128], warm_w[:], warm_w[:],
                                 start=True, stop=True)
            for gate_on in (d_wzx, d_xt0):
                for i in range(6):
                    wm = nc.tensor.matmul(warm_ps[:, :128], warm_w[:], warm_w[:],
                                          start=True, stop=True)
                    add_dep_helper(wm.ins, gate_on.ins, sync=True,
                                   reason="paced PE warmup")

            def wtile(wname, k, mi):
                off = (k * MT + mi) * 128
                return w_sb[wname][:, off:off + 128]

            def hview(h_m):
                # DVE can read bf16 directly; float32r tiles need a f32
                # bitcast for DVE consumption.
                return h_m[:] if MM_DT != "fp32r" else h_m[:].bitcast(f32)

            h_prev = [None] * MT

            # PSUM bank-pair sets, recreated per step via tags (6 data
            # banks + warm = 7 of 8). Accumulation groups are tracked per
            # BANK (zero region): exactly one start=True (first MM into
            # the bank) and one stop=True (last MM into the bank) even
            # though the two m-halves are separate output regions.
            zb_t, rb_t, gb_t = {}, {}, {}

            def half(banks, mi):
                return banks[mi // 2][:, (mi % 2) * NB:(mi % 2 + 1) * NB]

            def emit_x(t, gate_name):
                # x-projection block for one gate of step t (no recurrence
                # dependency; gated only by the weight DMA, the x tile DMA
                # and PSUM bank reuse)
                xt_t = get_xt(t)
                wname = {"z": "wzx", "r": "wrx", "g": "whx"}[gate_name]
                banks_map = {"z": zb_t, "r": rb_t, "g": gb_t}[gate_name]
                tag = {"z": "zb", "r": "rb", "g": "gb"}[gate_name]
                banks = [psum.tile([128, 512], f32, name=f"{gate_name}{t}_{i}",
                                   tag=f"{tag}{i}") for i in range(2)]
                banks_map[t] = banks
                last = t == 0 and gate_name in ("z", "g")
                for mi in range(MT):
                    for k in range(KT):
                        nc.tensor.matmul(half(banks, mi), wtile(wname, k, mi),
                                         xt_t[:, k * NB:(k + 1) * NB],
                                         start=(mi % 2 == 0 and k == 0),
                                         stop=(last and mi % 2 == 1 and k == KT - 1))

            emit_x(0, "z")
            emit_x(0, "g")

            for t in range(T):
                # x-projections for z(t) were emitted during step t-1
                # (between the Wrh and Whh blocks — they cover the PE's
                # ~1.1us wait for the r-gate ACT + r*h product)
                if t > 0:
                    emit_x(t, "r")
                    emit_x(t, "g")
                zb = zb_t.pop(t)
                gb = gb_t.pop(t)
                rb = rb_t.pop(t, None)

                # --- recurrent parts. Elementwise ops stay per m-tile
                # [128, NB]: the step-boundary critical path
                # (g ACT -> blend -> h -> next step's zh/rh matmuls) needs
                # h m-tiles to complete one at a time; [128, 2NB] pair ops
                # delayed h by ~0.5us/step (measured).
                z_t, g_t, h_t = [], [], []

                def emit_z_acts():
                    for mi in range(MT):
                        z_m = ssb.tile([128, NB], f32, name=f"z{t}m{mi}", tag=f"z{mi}")
                        nc.scalar.activation(z_m[:], half(zb, mi), Sig,
                                             bias=b_sb["bz"][:, mi:mi + 1])
                        z_t.append(z_m)

                if t > 0:
                    # k-outer: the k=0,1 matmuls for all m-tiles run first,
                    # giving the DVE blend of h m2/m3 (previous step) ~1us
                    # more slack before the k=2,3 matmuls consume them.
                    for k in range(KT):
                        for mi in range(MT):
                            nc.tensor.matmul(half(zb, mi), wtile("wzh", k, mi),
                                             h_prev[k][:],
                                             start=False,
                                             stop=(k == KT - 1 and mi % 2 == 1))
                    # z ACTs right away: they run during the Wrh block and
                    # free the zb banks for the hoisted x-z of step t+1
                    emit_z_acts()
                    for k in range(KT):
                        for mi in range(MT):
                            nc.tensor.matmul(half(rb, mi), wtile("wrh", k, mi),
                                             h_prev[k][:],
                                             start=False,
                                             stop=(k == KT - 1 and mi % 2 == 1))

                    # r gate first (feeds r*h -> Whh matmuls)
                    r_t, rh_t = [], []
                    for mi in range(MT):
                        r_m = ssb.tile([128, NB], f32, name=f"r{t}m{mi}", tag=f"r{mi}")
                        nc.scalar.activation(r_m[:], half(rb, mi), Sig,
                                             bias=b_sb["br"][:, mi:mi + 1])
                        r_t.append(r_m)
                    # Hoisted x-z of step t+1: in PE issue order this block
                    # sits between Wrh and Whh, covering the PE's ~1.1us
                    # wait for r ACT + r*h (measured ~190ns/step stall
                    # otherwise; GPSIMD rh muls made it worse).
                    if t + 1 < T:
                        emit_x(t + 1, "z")
                    for mi in range(MT):
                        rh_m = ssb.tile([128, NB], mdt, name=f"rh{t}m{mi}", tag=f"rh{mi}")
                        nc.vector.tensor_mul(rh_m[:], r_t[mi][:], hview(h_prev[mi]))
                        rh_t.append(rh_m)

                    for k in range(KT):
                        for mi in range(MT):
                            nc.tensor.matmul(half(gb, mi), wtile("whh", k, mi),
                                             rh_t[k][:],
                                             start=False,
                                             stop=(k == KT - 1 and mi % 2 == 1))
                else:
                    emit_z_acts()
                    if T > 1:
                        emit_x(1, "z")

                for mi in range(MT):
                    g_m = ssb.tile([128, NB], f32, name=f"g{t}m{mi}", tag=f"g{mi}")
                    nc.scalar.activation(g_m[:], half(gb, mi), Tanh,
                                         bias=b_sb["bh"][:, mi:mi + 1])
                    g_t.append(g_m)

                for mi in range(MT):
                    h_m = hsb.tile([128, NB], mdt, name=f"h{t}m{mi}", tag=f"h{mi}")
                    tmp = ssb.tile([128, NB], f32, name=f"tmp{t}m{mi}", tag=f"tmp{mi}")
                    # All blend chains stay on DVE. Offloading any of them
                    # (or the rh muls) to GPSIMD measured WORSE: its
                    # ~670ns ops + cross-engine semaphore latency feed the
                    # recurrence directly and stall the next step's
                    # matmuls far more than the ~244ns they were hiding.
                    if t == 0:
                        # h = (1 - z) * g = g - z*g
                        nc.vector.tensor_mul(tmp[:], z_t[mi][:], g_t[mi][:])
                        nc.vector.tensor_sub(h_m[:], g_t[mi][:], tmp[:])
                    else:
                        # h = g + z*(h_prev - g)
                        nc.vector.tensor_sub(tmp[:], hview(h_prev[mi]), g_t[mi][:])
                        nc.vector.tensor_mul(tmp[:], tmp[:], z_t[mi][:])
                        nc.vector.tensor_add(h_m[:], g_t[mi][:], tmp[:])
                    h_t.append(h_m)
                    nc.sync.dma_start(
                        out=out_d[t, mi * 128:(mi + 1) * 128, :],
                        in_=h_m[:] if MM_DT == "bf16" else h_m[:].bitcast(f32),
                    )
                h_prev = h_t

    nc.compile()
    return nc


def _get_nc():
    key = MM_DT
    if key not in _CACHE:
        _CACHE[key] = _build_nc()
    return _CACHE[key]


def _mm_np_dtype():
    if MM_DT == "bf16":
        import ml_dtypes
        return np.dtype(ml_dtypes.bfloat16)
    return np.dtype(np.float32)


def _pack_w(W):
    # W [dout, din] -> lhsT tiles packed [128, KT*MT*128], tile (k,m) at
    # free offset (k*MT+m)*128: w[p, off+q] = W[m*128+q, k*128+p]
    Wt = np.asarray(W, np.float32).T.reshape(KT, 128, MT, 128)
    packed = np.ascontiguousarray(Wt.transpose(1, 0, 2, 3).reshape(128, KT * MT * 128))
    return packed.astype(_mm_np_dtype())


def kernel(inputss, Wzx, Wzh, Wrx, Wrh, Whx, Whh, bz, br, bh):
    global LAST_RESULT
    from concourse.bass_utils import run_bass_kernel_spmd

    inputss = np.asarray(inputss, np.float32)
    assert inputss.shape == (N, T, D), inputss.shape

    # host-side shard + layout prep: [NC, T, p(128), k(KT), b(NB)] so the
    # per-step device DMA is fully contiguous
    xs = (inputss.reshape(NC, NB, T, KT, 128)
          .transpose(0, 2, 4, 3, 1)
          .reshape(NC, T, 128, KT * NB))
    wp = {"wzx": _pack_w(Wzx), "wzh": _pack_w(Wzh),
          "wrx": _pack_w(Wrx), "wrh": _pack_w(Wrh),
          "whx": _pack_w(Whx), "whh": _pack_w(Whh)}
    bp = {"bz": np.ascontiguousarray(np.asarray(bz, np.float32).reshape(MT, 128).T),
          "br": np.ascontiguousarray(np.asarray(br, np.float32).reshape(MT, 128).T),
          "bh": np.ascontiguousarray(np.asarray(bh, np.float32).reshape(MT, 128).T)}

    mmdt = _mm_np_dtype()
    in_maps = []
    for c in range(NC):
        m = {"xt": np.ascontiguousarray(xs[c]).astype(mmdt)}
        m.update(wp)
        m.update(bp)
        in_maps.append(m)

    nc = _get_nc()
    trace = bool(int(os.environ.get("GRU_TRACE", "0")))
    res = run_bass_kernel_spmd(nc, in_maps, core_ids=list(range(NC)), trace=trace)
    LAST_RESULT = res

    outs = np.stack([np.asarray(res.results[c]["out"], np.float32)
                     for c in range(NC)])  # [NC, T, D, NB]
    return np.ascontiguousarray(outs.transpose(0, 3, 1, 2).reshape(N, T, D))
